# revision 43
# baseline (speedup 1.0000x reference)
# Deformable-conv (DCNv2-style, scrambled-reshape variant) Trainium2 Bass kernel.
# Data-parallel over batch: 8 samples -> 8 NeuronCores.
#
# Per-core pipeline (layouts derived + validated against the reference):
#   1. offset conv (18ch) + modulation conv (9ch) in ONE fp16 pass over padded x.
#      The mod conv runs on the transposed image via a transposed access
#      pattern on the same xpad tile (no second image needed); outputs land in
#      one [27, 4096] fp16 tile (rows 0:18 offsets, 18:27 sigmoid(mod)).
#   2. PE "transposes" (regular fp16 matmuls vs identity) to pixel-major
#      [128 pix, 32 chunk, 27].
#   3. Per kernel-point n2: 3 host-constant selection matmuls pick the
#      (source-pixel, source-channel) pair per partition; pointwise metadata
#      (DVE) produces a flat 2x2-patch row index + 4 bilinear*modulation
#      scales (fp16, corner-innermost).
#   4. Indirect-DMA gathers from a host-built patch table whose rows are
#      channel-outer/corner-inner (row f = [c0:4 corners, c1:4 corners, ...]),
#      so the scale multiply has packed fp16 innermost dims on every operand
#      (DVE 2x mode). One [128,1]-offset gather per (n2, chunk).
#   5. One DVE mul (scales) + one DVE pair-add (4 corners -> 2), then the
#      final corner reduction + transpose to channel-major happen on the PE:
#      2 PSUM-accumulated matmuls per 128-pixel chunk against identity.
#   6. Main conv = 9 accumulated fp16 matmuls per 512-pixel block; PSUM
#      copies write through a transposed AP into a full-row [128, 4096]
#      staging tile, stored with one contiguous DMA per 128-channel half.
import sys

import numpy as np

sys.path.insert(0, "/opt/trn_rl_repo")

import concourse.bass as bass
import concourse.bacc as bacc
import concourse.mybir as mybir
from concourse import tile
from concourse.bass_utils import run_bass_kernel_spmd

F32 = mybir.dt.float32
F16 = mybir.dt.float16
I32 = mybir.dt.int32

B, C, H, W = 8, 128, 64, 64
OUT = 256
PIX = H * W            # 4096
KCH = 32               # pixel-major chunks (4096 / 128)
TROWS = 4224           # patch table rows (4096 + pad for f+65 reads)

_CACHE = {}


def _build_host_constants():
    if "sel" in _CACHE:
        return _CACHE
    p2 = np.arange(128)
    k2 = np.arange(KCH)
    sel = np.zeros((128, 9, 3, 128), np.float16)   # [p_src, n2, r, p2]
    basey = np.zeros((128, 9, KCH), np.float32)    # [p, n2, k]
    basex = np.zeros((128, 9, KCH), np.float32)
    for n2 in range(9):
        a2, e2 = n2 // 3, n2 % 3
        i2 = p2 % 64
        r = (i2 + e2) % 3
        n = 3 * r + a2                       # source kernel point per partition
        J = (64 * e2 + i2) // 3              # source col j per partition
        c_src = 64 * (p2 // 64) + J          # source partition in pixel-major
        for rr in range(3):
            m = r == rr
            sel[c_src[m], n2, rr, p2[m]] = 1.0
        a = n // 3
        e = n % 3
        # y_u = i + a + o_y ; i = j2 = 2*k2 + p2//64
        basey[:, n2, :] = (2 * k2[None, :] + (p2 // 64)[:, None]) + a[:, None]
        basex[:, n2, :] = (J + e)[:, None] * np.ones((1, KCH), np.float32)
    _CACHE["sel"] = sel
    _CACHE["basyx"] = np.ascontiguousarray(np.stack([basey, basex], axis=-1))
    _CACHE["ident16"] = np.eye(128, dtype=np.float16)
    return _CACHE


def _pad66(img):  # [C,64,64] -> [C, 66*66] zero-padded fp16
    p = np.zeros((C, 66, 66), np.float16)
    p[:, 1:65, 1:65] = img
    return p.reshape(C, 66 * 66)


def _patch_table(img):  # [C,64,64] f32 -> [TROWS, 512] fp16, channel-outer rows
    flat = np.zeros((C, TROWS + 65), np.float16)
    flat[:, :PIX] = img.reshape(C, PIX).astype(np.float16)
    f = np.arange(TROWS)
    tab = np.stack(
        [flat[:, f], flat[:, f + 1], flat[:, f + 64], flat[:, f + 65]], axis=-1
    )  # [C, TROWS, 4]
    return np.ascontiguousarray(tab.transpose(1, 0, 2)).reshape(TROWS, 512)


def _build_program():
    if "nc" in _CACHE:
        return _CACHE["nc"]
    nc = bacc.Bacc()
    d = {}
    d["xpad"] = nc.dram_tensor("xpad", [C, 66 * 66], F16, kind="ExternalInput")
    d["ptab"] = nc.dram_tensor("ptab", [TROWS, 512], F16, kind="ExternalInput")
    d["womb"] = nc.dram_tensor("womb", [C, 9, 18], F16, kind="ExternalInput")
    d["wmtb"] = nc.dram_tensor("wmtb", [C, 9, 9], F16, kind="ExternalInput")
    d["ob"] = nc.dram_tensor("ob", [18, 1], F32, kind="ExternalInput")
    d["mb"] = nc.dram_tensor("mb", [9, 1], F32, kind="ExternalInput")
    d["selt"] = nc.dram_tensor("selt", [128, 9 * 3 * 128], F16, kind="ExternalInput")
    d["basyx"] = nc.dram_tensor("basyx", [128, 9 * KCH * 2], F32,
                                kind="ExternalInput")
    d["w2"] = nc.dram_tensor("w2", [C, 9 * 2 * 128], F16, kind="ExternalInput")
    d["id16"] = nc.dram_tensor("id16", [128, 128], F16, kind="ExternalInput")
    # raw main-conv PSUM blocks [sq*4 + hf*2 + q] = [128 out-ch, 512 pi2'];
    # the fixed pi2'->pixel permutation happens on host during unshard
    d["outr"] = nc.dram_tensor("outr", [16, 128, 512], F32, kind="ExternalOutput")

    AO = mybir.AluOpType

    with tile.TileContext(nc) as tc:
        with (
            tc.tile_pool(name="imgs", bufs=1) as imgs,
            tc.tile_pool(name="wts", bufs=1) as wts,
            tc.tile_pool(name="meta", bufs=1) as meta,
            tc.tile_pool(name="gbuf", bufs=8) as gbuf,
            tc.tile_pool(name="hbuf", bufs=2) as hbuf,
            tc.tile_pool(name="vbuf", bufs=2) as vbuf,
            tc.tile_pool(name="obuf", bufs=4) as obuf,
        ):
            # ---- load image + weights + constants (single DMAs each; order =
            #      first-use order so the conv pipeline starts ASAP)
            # loads ordered by first use on the critical path: the offsets
            # conv (xpad rows 0:10 + womb) unblocks first, big slices later
            xpad = imgs.tile([C, 66 * 66], F16)
            nc.sync.dma_start(xpad[:, 0:660], d["xpad"][:, 0:660])
            womb = wts.tile([C, 9, 18], F16)
            nc.sync.dma_start(womb[:], d["womb"][:])
            ob = wts.tile([18, 1], F32)
            nc.sync.dma_start(ob[:], d["ob"][:])
            id16 = wts.tile([128, 128], F16)
            nc.sync.dma_start(id16[:], d["id16"][:])
            selt = wts.tile([128, 9, 3, 128], F16)
            nc.sync.dma_start(selt[:], d["selt"][:])
            basyx = wts.tile([128, 9, KCH, 2], F32)
            nc.sync.dma_start(basyx[:], d["basyx"][:])
            nc.sync.dma_start(xpad[:, 660:66 * 34], d["xpad"][:, 660:66 * 34])
            wmtb = wts.tile([C, 9, 9], F16)
            nc.sync.dma_start(wmtb[:], d["wmtb"][:])
            mb = wts.tile([9, 1], F32)
            nc.sync.dma_start(mb[:], d["mb"][:])
            nc.sync.dma_start(xpad[:, 66 * 34:], d["xpad"][:, 66 * 34:])
            w2 = wts.tile([C, 9, 2, 128], F16)
            nc.sync.dma_start(w2[:], d["w2"][:])
            # junk tiles for PE p-state warmup + Act table preload (values
            # never consumed)
            junka = wts.tile([128, 128], F16)
            junkb = wts.tile([128, 512], F16)
            junkc = wts.tile([18, 4], F16)
            nc.vector.memset(junka[:], 0.0)
            nc.vector.memset(junkb[:], 0.0)
            nc.scalar.activation(junkc[:], junka[0:18, 0:4],
                                 mybir.ActivationFunctionType.Identity,
                                 bias=0.0, scale=1.0)
            nc.scalar.activation(junkc[:], junka[0:18, 0:4],
                                 mybir.ActivationFunctionType.Sigmoid,
                                 bias=0.0, scale=1.0)

            # rows 0:18 offsets, 32:41 mod (engine outputs need 32-aligned
            # partition starts; rows 18:32 stay uninitialized and are never
            # read -- the transposes contract only 0:18 / 32:41)
            ocm = meta.tile([41, PIX], F16)
            opm = meta.tile([128, KCH, 41], F16)   # pixel-major
            scal = meta.tile([128, 9, KCH, 4], F16)
            idxt = meta.tile([128, 9, KCH], I32)

            with (
                tc.tile_pool(name="psc", bufs=2, space="PSUM") as psc,
                tc.tile_pool(name="pst1", bufs=2, space="PSUM") as pst1,
            ):
                # PE p-state warmup: junk matmuls keep the PE busy from t=0 so
                # the conv matmuls run at full clock once xpad lands
                warm = psc.tile([18, 512], F32, tag="po", name="warm")
                for _ in range(4):
                    nc.tensor.matmul(warm[:], junka[:, 0:18], junkb[:],
                                     start=True, stop=True)

                # Front-end in two phases: a small first phase (conv tiles
                # 0:2, chunks 0:8) so the first gathers start early, then the
                # rest in one pass (fewer small DVE metadata ops).
                # Within each phase the offsets path (po -> trA -> sel -> idx)
                # is emitted before the mod path so gathers never wait on the
                # mod conv.
                for tl_lo, tl_hi, k_lo, k_hi in ((0, 2, 0, 8), (2, 8, 8, 32)):
                    # ---- conv1 (offsets) + conv2 (mod, via transposed read)
                    for tl in range(tl_lo, tl_hi):
                        po = psc.tile([18, 512], F32, tag="po")
                        pm = psc.tile([9, 512], F32, tag="pm")
                        for t in range(9):
                            dy, dx = t // 3, t % 3
                            rhs1 = bass.AP(
                                tensor=xpad[:].tensor,
                                offset=xpad[:].offset + dy * 66 + dx + tl * 8 * 66,
                                ap=[list(xpad[:].ap[0]), [66, 8], [1, 64]],
                            )
                            nc.tensor.matmul(po[:], womb[:, t, :], rhs1,
                                             start=(t == 0), stop=(t == 8))
                        for t in range(9):
                            dy, dx = t // 3, t % 3
                            rhs2 = bass.AP(
                                tensor=xpad[:].tensor,
                                offset=xpad[:].offset + dx * 66 + dy + tl * 8,
                                ap=[list(xpad[:].ap[0]), [1, 8], [66, 64]],
                            )
                            nc.tensor.matmul(pm[:], wmtb[:, t, :], rhs2,
                                             start=(t == 0), stop=(t == 8))
                        nc.scalar.activation(
                            ocm[0:18, tl * 512:(tl + 1) * 512], po[:],
                            mybir.ActivationFunctionType.Identity,
                            bias=ob[:], scale=1.0)
                        nc.scalar.activation(
                            ocm[32:41, tl * 512:(tl + 1) * 512], pm[:],
                            mybir.ActivationFunctionType.Sigmoid,
                            bias=mb[:], scale=1.0)

                    # ---- PE-transpose conv outputs to pixel-major; offsets
                    #      (partitions 0:18) and mod (32:41) transposed
                    #      separately so the offsets copy never waits on mod
                    for tl in range(tl_lo, tl_hi):
                        ptA = pst1.tile([128, 4, 18], F32, tag="ptA", bufs=1)
                        ptB = pst1.tile([128, 4, 9], F32, tag="ptB", bufs=1)
                        for k4 in range(4):
                            k = tl * 4 + k4
                            nc.tensor.matmul(
                                ptA[:, k4, :],
                                ocm[0:18, k * 128:(k + 1) * 128],
                                id16[0:18, 0:18], start=True, stop=True)
                        for k4 in range(4):
                            k = tl * 4 + k4
                            nc.tensor.matmul(
                                ptB[:, k4, :],
                                ocm[32:41, k * 128:(k + 1) * 128],
                                id16[32:41, 32:41], start=True, stop=True)
                        dstA = bass.AP(
                            tensor=opm[:].tensor,
                            offset=opm[:].offset + tl * 4 * 41,
                            ap=[list(opm[:].ap[0]), [41, 4], [1, 18]],
                        )
                        nc.vector.tensor_copy(dstA, ptA[:])
                        dstB = bass.AP(
                            tensor=opm[:].tensor,
                            offset=opm[:].offset + tl * 4 * 41 + 32,
                            ap=[list(opm[:].ap[0]), [41, 4], [1, 9]],
                        )
                        nc.vector.tensor_copy(dstB, ptB[:])

                    # ---- per-n2 metadata -> idx + scales (this phase's chunks)
                    HK = k_hi - k_lo
                    ho = k_lo
                    for n2 in range(9):
                        a2 = n2 // 3
                        oyx = pst1.tile([128, HK, 2], F32, tag="sel")
                        for r in range(3):
                            ch = 3 * r + a2
                            rhs = bass.AP(
                                tensor=opm[:].tensor,
                                offset=opm[:].offset + ch + ho * 41,
                                ap=[list(opm[:].ap[0]), [41, HK], [9, 2]],
                            )
                            nc.tensor.matmul(oyx[:], selt[:, n2, r, :], rhs,
                                             start=(r == 0), stop=(r == 2))
                        P = meta.tile([128, HK, 2], F32, tag="P")
                        nc.vector.tensor_add(P[:], oyx[:],
                                             basyx[:, n2, ho:ho + HK, :])
                        nc.vector.tensor_scalar(P[:], P[:], 0.0, 63.0,
                                                AO.max, AO.min)
                        R0 = meta.tile([128, HK, 2], F32, tag="R0")
                        nc.vector.tensor_scalar(R0[:], P[:], -0.5, 12582912.0,
                                                AO.add, AO.add)
                        nc.vector.tensor_scalar_add(R0[:], R0[:], -12582912.0)
                        F = meta.tile([128, HK, 2], F32, tag="F")
                        nc.vector.tensor_sub(F[:], P[:], R0[:])
                        f00 = meta.tile([128, HK], F32, tag="f00")
                        nc.vector.scalar_tensor_tensor(
                            f00[:], R0[:, :, 1], 64.0, R0[:, :, 0], AO.mult, AO.add)
                        nc.vector.tensor_copy(idxt[:, n2, ho:ho + HK], f00[:])
                        mrow = opm[:, ho:ho + HK, 32 + n2]
                        v1 = meta.tile([128, HK], F32, tag="v1")
                        v0 = meta.tile([128, HK], F32, tag="v0")
                        sc4 = meta.tile([128, 4, HK], F32, tag="sc4")
                        nc.vector.tensor_mul(v1[:], mrow, F[:, :, 1])
                        nc.vector.tensor_sub(v0[:], mrow, v1[:])
                        nc.vector.tensor_mul(sc4[:, 1, :], v0[:], F[:, :, 0])
                        nc.vector.tensor_sub(sc4[:, 0, :], v0[:], sc4[:, 1, :])
                        nc.vector.tensor_mul(sc4[:, 3, :], v1[:], F[:, :, 0])
                        nc.vector.tensor_sub(sc4[:, 2, :], v1[:], sc4[:, 3, :])
                        # convert to fp16 [k, corner]-interleaved in one copy
                        csrc = bass.AP(
                            tensor=sc4[:].tensor, offset=sc4[:].offset,
                            ap=[list(sc4[:].ap[0]), [1, HK], [HK, 4]],
                        )
                        cdst = bass.AP(
                            tensor=scal[:].tensor,
                            offset=scal[:].offset + n2 * (KCH * 4) + ho * 4,
                            ap=[list(scal[:].ap[0]), [4, HK], [1, 4]],
                        )
                        nc.vector.tensor_copy(cdst, csrc)

            with (
                tc.tile_pool(name="pst", bufs=3, space="PSUM") as pst,
                tc.tile_pool(name="psm", bufs=1, space="PSUM") as psm,
            ):
                # ---- per spatial-quarter: gather + scale + reduce-transpose;
                #      main-conv matmuls interleave per n2 (PSUM accumulates
                #      while later n2 groups are still gathering)
                for sq in range(4):
                    vc = vbuf.tile([C, 9, 1024], F16, tag="vc")
                    accq = {}
                    for hf in range(2):
                        for tl2 in range(2):
                            accq[hf, tl2] = psm.tile(
                                [128, 512], F32, tag=f"mm{hf}{tl2}",
                                name=f"acc{hf}{tl2}")
                    for n2 in range(9):
                        g = gbuf.tile([128, 8, 512], F16, tag="g")
                        for kk in range(8):
                            k = sq * 8 + kk
                            dstg = bass.AP(
                                tensor=g[:].tensor,
                                offset=g[:].offset + kk * 512,
                                ap=[list(g[:].ap[0]), [1, 512]],
                            )
                            nc.gpsimd.indirect_dma_start(
                                out=dstg, out_offset=None,
                                in_=d["ptab"][:],
                                in_offset=bass.IndirectOffsetOnAxis(
                                    ap=idxt[:, n2, k:k + 1], axis=0),
                            )
                        h = hbuf.tile([128, 8, 128, 2], F16, tag="h")
                        for q in range(2):
                            gv = bass.AP(
                                tensor=g[:].tensor,
                                offset=g[:].offset + q * 2048,
                                ap=[list(g[:].ap[0]), [512, 4], [4, 128], [1, 4]],
                            )
                            sv = bass.AP(
                                tensor=scal[:].tensor,
                                offset=(scal[:].offset + n2 * (KCH * 4)
                                        + sq * 32 + q * 16),
                                ap=[list(scal[:].ap[0]), [4, 4], [0, 128], [1, 4]],
                            )
                            nc.vector.tensor_mul(gv, gv, sv)
                            ha = bass.AP(
                                tensor=g[:].tensor,
                                offset=g[:].offset + q * 2048,
                                ap=[list(g[:].ap[0]), [512, 4], [4, 128], [1, 2]],
                            )
                            hb = bass.AP(
                                tensor=g[:].tensor,
                                offset=g[:].offset + q * 2048 + 2,
                                ap=[list(g[:].ap[0]), [512, 4], [4, 128], [1, 2]],
                            )
                            hd = bass.AP(
                                tensor=h[:].tensor,
                                offset=h[:].offset + q * 1024,
                                ap=[list(h[:].ap[0]), [256, 4], [2, 128], [1, 2]],
                            )
                            nc.vector.tensor_add(hd, ha, hb)
                            acc = pst.tile([128, 512], F32, tag="tr")
                            for kk4 in range(4):
                                kk = q * 4 + kk4
                                for j in range(2):
                                    lhsT = bass.AP(
                                        tensor=h[:].tensor,
                                        offset=h[:].offset + kk * 256 + j,
                                        ap=[list(h[:].ap[0]), [2, 128]],
                                    )
                                    nc.tensor.matmul(
                                        acc[:, kk4 * 128:(kk4 + 1) * 128],
                                        lhsT, id16[:],
                                        start=(j == 0), stop=(j == 1))
                            nc.scalar.copy(vc[:, n2, q * 512:(q + 1) * 512], acc[:])
                            for hf in range(2):
                                nc.tensor.matmul(
                                    accq[hf, q][:], w2[:, n2, hf, :],
                                    vc[:, n2, q * 512:(q + 1) * 512],
                                    start=(n2 == 0), stop=(n2 == 8))

                    # store raw blocks (contiguous; host unscrambles)
                    for hf in range(2):
                        for q in range(2):
                            outq = obuf.tile([128, 512], F32, tag="oq",
                                             name="outq")
                            nc.scalar.copy(outq[:], accq[hf, q][:])
                            nc.sync.dma_start(
                                d["outr"][sq * 4 + hf * 2 + q], outq[:])

    nc.compile()
    _CACHE["nc"] = nc
    return nc


def _host_inputs(b_x, offset_w, offset_b, mod_w, mod_b, conv_w):
    hc = _build_host_constants()
    img = b_x.astype(np.float32)
    womb = np.zeros((C, 9, 18), np.float16)
    wmtb = np.zeros((C, 9, 9), np.float16)
    for t in range(9):
        dy, dx = t // 3, t % 3
        womb[:, t, :] = offset_w[:, :, dy, dx].T
        wmtb[:, 3 * dx + dy, :] = mod_w[:, :, dy, dx].T
    w2 = np.zeros((C, 9, 2, 128), np.float16)
    for n2 in range(9):
        a2, e2 = n2 // 3, n2 % 3
        for hf in range(2):
            w2[:, n2, hf, :] = conv_w[128 * hf:128 * (hf + 1), :, a2, e2].T
    return {
        "xpad": _pad66(img),
        "ptab": _patch_table(img),
        "womb": womb,
        "wmtb": wmtb,
        "ob": offset_b.reshape(18, 1).astype(np.float32),
        "mb": mod_b.reshape(9, 1).astype(np.float32),
        "selt": hc["sel"].reshape(128, 9 * 3 * 128),
        "basyx": hc["basyx"].reshape(128, 9 * KCH * 2),
        "w2": w2.reshape(C, 9 * 2 * 128),
        "id16": hc["ident16"],
    }


def kernel(x, offset_w, offset_b, mod_w, mod_b, conv_w):
    nc = _build_program()
    in_maps = [
        _host_inputs(x[b], offset_w, offset_b, mod_w, mod_b, conv_w)
        for b in range(B)
    ]
    res = run_bass_kernel_spmd(nc, in_maps, core_ids=list(range(B)))
    out = np.empty((B, OUT, H, W), np.float32)
    for b in range(B):
        # outr[sq*4 + hf*2 + q] = [128 o, 512 pi2'] with
        # pi2' = (2sq+q)*512 + q2, j2 = 8*(2sq+q) + q2//64, i2 = q2%64
        outr = res.results[b]["outr"].reshape(4, 2, 2, 128, 8, 64)
        for sq in range(4):
            for hf in range(2):
                for q in range(2):
                    j2 = 16 * sq + 8 * q
                    out[b, 128 * hf:128 * (hf + 1), :, j2:j2 + 8] = (
                        outr[sq, hf, q].transpose(0, 2, 1))
    return out


if __name__ == "__main__":
    rng = np.random.default_rng(0)
    ins = {
        "x": rng.standard_normal((B, C, H, W), dtype=np.float32),
        "offset_w": (rng.standard_normal((18, C, 3, 3)) / 34).astype(np.float32),
        "offset_b": (rng.standard_normal(18) * 0.01).astype(np.float32),
        "mod_w": (rng.standard_normal((9, C, 3, 3)) / 34).astype(np.float32),
        "mod_b": (rng.standard_normal(9) * 0.01).astype(np.float32),
        "conv_w": (rng.standard_normal((OUT, C, 3, 3)) / 34).astype(np.float32),
    }
    o = kernel(**ins)
    print("out", o.shape, o.dtype, np.abs(o).max())


# revision 44
# speedup vs baseline: 1.0121x; 1.0121x over previous
# Deformable-conv (DCNv2-style, scrambled-reshape variant) Trainium2 Bass kernel.
# Data-parallel over batch: 8 samples -> 8 NeuronCores.
#
# Per-core pipeline (layouts derived + validated against the reference):
#   1. offset conv (18ch) + modulation conv (9ch) in ONE fp16 pass over padded x.
#      The mod conv runs on the transposed image via a transposed access
#      pattern on the same xpad tile (no second image needed); outputs land in
#      one [27, 4096] fp16 tile (rows 0:18 offsets, 18:27 sigmoid(mod)).
#   2. PE "transposes" (regular fp16 matmuls vs identity) to pixel-major
#      [128 pix, 32 chunk, 27].
#   3. Per kernel-point n2: 3 host-constant selection matmuls pick the
#      (source-pixel, source-channel) pair per partition; pointwise metadata
#      (DVE) produces a flat 2x2-patch row index + 4 bilinear*modulation
#      scales (fp16, corner-innermost).
#   4. Indirect-DMA gathers from a host-built patch table whose rows are
#      channel-outer/corner-inner (row f = [c0:4 corners, c1:4 corners, ...]),
#      so the scale multiply has packed fp16 innermost dims on every operand
#      (DVE 2x mode). One [128,1]-offset gather per (n2, chunk).
#   5. One DVE mul (scales) + one DVE pair-add (4 corners -> 2), then the
#      final corner reduction + transpose to channel-major happen on the PE:
#      2 PSUM-accumulated matmuls per 128-pixel chunk against identity.
#   6. Main conv = 9 accumulated fp16 matmuls per 512-pixel block; PSUM
#      copies write through a transposed AP into a full-row [128, 4096]
#      staging tile, stored with one contiguous DMA per 128-channel half.
import sys

import numpy as np

sys.path.insert(0, "/opt/trn_rl_repo")

import concourse.bass as bass
import concourse.bacc as bacc
import concourse.mybir as mybir
from concourse import tile
from concourse.bass_utils import run_bass_kernel_spmd

F32 = mybir.dt.float32
F16 = mybir.dt.float16
I32 = mybir.dt.int32

B, C, H, W = 8, 128, 64, 64
OUT = 256
PIX = H * W            # 4096
KCH = 32               # pixel-major chunks (4096 / 128)
TROWS = 4224           # patch table rows (4096 + pad for f+65 reads)

_CACHE = {}


def _build_host_constants():
    if "sel" in _CACHE:
        return _CACHE
    p2 = np.arange(128)
    k2 = np.arange(KCH)
    sel = np.zeros((128, 9, 3, 128), np.float16)   # [p_src, n2, r, p2]
    basey = np.zeros((128, 9, KCH), np.float32)    # [p, n2, k]
    basex = np.zeros((128, 9, KCH), np.float32)
    for n2 in range(9):
        a2, e2 = n2 // 3, n2 % 3
        i2 = p2 % 64
        r = (i2 + e2) % 3
        n = 3 * r + a2                       # source kernel point per partition
        J = (64 * e2 + i2) // 3              # source col j per partition
        c_src = 64 * (p2 // 64) + J          # source partition in pixel-major
        for rr in range(3):
            m = r == rr
            sel[c_src[m], n2, rr, p2[m]] = 1.0
        a = n // 3
        e = n % 3
        # y_u = i + a + o_y ; i = j2 = 2*k2 + p2//64
        basey[:, n2, :] = (2 * k2[None, :] + (p2 // 64)[:, None]) + a[:, None]
        basex[:, n2, :] = (J + e)[:, None] * np.ones((1, KCH), np.float32)
    _CACHE["sel"] = sel
    _CACHE["basyx"] = np.ascontiguousarray(np.stack([basey, basex], axis=-1))
    _CACHE["ident16"] = np.eye(128, dtype=np.float16)
    return _CACHE


def _pad66(img):  # [C,64,64] -> [C, 66*66] zero-padded fp16
    p = np.zeros((C, 66, 66), np.float16)
    p[:, 1:65, 1:65] = img
    return p.reshape(C, 66 * 66)


def _patch_table(img):  # [C,64,64] f32 -> [TROWS, 512] fp16, channel-outer rows
    flat = np.zeros((C, TROWS + 65), np.float16)
    flat[:, :PIX] = img.reshape(C, PIX).astype(np.float16)
    f = np.arange(TROWS)
    tab = np.stack(
        [flat[:, f], flat[:, f + 1], flat[:, f + 64], flat[:, f + 65]], axis=-1
    )  # [C, TROWS, 4]
    return np.ascontiguousarray(tab.transpose(1, 0, 2)).reshape(TROWS, 512)


def _build_program():
    if "nc" in _CACHE:
        return _CACHE["nc"]
    nc = bacc.Bacc()
    d = {}
    d["xpad"] = nc.dram_tensor("xpad", [C, 66 * 66], F16, kind="ExternalInput")
    d["ptab"] = nc.dram_tensor("ptab", [TROWS, 512], F16, kind="ExternalInput")
    d["womb"] = nc.dram_tensor("womb", [C, 9, 18], F16, kind="ExternalInput")
    d["wmtb"] = nc.dram_tensor("wmtb", [C, 9, 9], F16, kind="ExternalInput")
    d["ob"] = nc.dram_tensor("ob", [18, 1], F32, kind="ExternalInput")
    d["mb"] = nc.dram_tensor("mb", [9, 1], F32, kind="ExternalInput")
    d["selt"] = nc.dram_tensor("selt", [128, 9 * 3 * 128], F16, kind="ExternalInput")
    d["basyx"] = nc.dram_tensor("basyx", [128, 9 * KCH * 2], F32,
                                kind="ExternalInput")
    d["w2"] = nc.dram_tensor("w2", [C, 9 * 2 * 128], F16, kind="ExternalInput")
    d["id16"] = nc.dram_tensor("id16", [128, 128], F16, kind="ExternalInput")
    # raw main-conv PSUM blocks [sq*4 + hf*2 + q] = [128 out-ch, 512 pi2'];
    # the fixed pi2'->pixel permutation happens on host during unshard
    d["outr"] = nc.dram_tensor("outr", [16, 128, 512], F32, kind="ExternalOutput")

    AO = mybir.AluOpType

    with tile.TileContext(nc) as tc:
        with (
            tc.tile_pool(name="imgs", bufs=1) as imgs,
            tc.tile_pool(name="wts", bufs=1) as wts,
            tc.tile_pool(name="meta", bufs=1) as meta,
            tc.tile_pool(name="gbuf", bufs=8) as gbuf,
            tc.tile_pool(name="hbuf", bufs=2) as hbuf,
            tc.tile_pool(name="vbuf", bufs=2) as vbuf,
            tc.tile_pool(name="obuf", bufs=4) as obuf,
        ):
            # ---- load image + weights + constants (single DMAs each; order =
            #      first-use order so the conv pipeline starts ASAP)
            # loads ordered by first use on the critical path: the offsets
            # conv (xpad rows 0:10 + womb) unblocks first, big slices later
            xpad = imgs.tile([C, 66 * 66], F16)
            nc.sync.dma_start(xpad[:, 0:660], d["xpad"][:, 0:660])
            womb = wts.tile([C, 9, 18], F16)
            nc.sync.dma_start(womb[:], d["womb"][:])
            ob = wts.tile([18, 1], F32)
            nc.sync.dma_start(ob[:], d["ob"][:])
            id16 = wts.tile([128, 128], F16)
            nc.sync.dma_start(id16[:], d["id16"][:])
            selt = wts.tile([128, 9, 3, 128], F16)
            nc.sync.dma_start(selt[:], d["selt"][:])
            basyx = wts.tile([128, 9, KCH, 2], F32)
            nc.sync.dma_start(basyx[:], d["basyx"][:])
            nc.sync.dma_start(xpad[:, 660:66 * 34], d["xpad"][:, 660:66 * 34])
            wmtb = wts.tile([C, 9, 9], F16)
            nc.sync.dma_start(wmtb[:], d["wmtb"][:])
            mb = wts.tile([9, 1], F32)
            nc.sync.dma_start(mb[:], d["mb"][:])
            nc.sync.dma_start(xpad[:, 66 * 34:], d["xpad"][:, 66 * 34:])
            w2 = wts.tile([C, 9, 2, 128], F16)
            nc.sync.dma_start(w2[:], d["w2"][:])
            # junk tiles for PE p-state warmup + Act table preload (values
            # never consumed)
            junka = wts.tile([128, 128], F16)
            junkb = wts.tile([128, 512], F16)
            junkc = wts.tile([18, 4], F16)
            nc.vector.memset(junka[:], 0.0)
            nc.vector.memset(junkb[:], 0.0)
            nc.scalar.activation(junkc[:], junka[0:18, 0:4],
                                 mybir.ActivationFunctionType.Identity,
                                 bias=0.0, scale=1.0)
            nc.scalar.activation(junkc[:], junka[0:18, 0:4],
                                 mybir.ActivationFunctionType.Sigmoid,
                                 bias=0.0, scale=1.0)

            # rows 0:18 offsets, 32:41 mod (engine outputs need 32-aligned
            # partition starts; rows 18:32 stay uninitialized and are never
            # read -- the transposes contract only 0:18 / 32:41)
            ocm = meta.tile([41, PIX], F16)
            opm = meta.tile([128, KCH, 41], F16)   # pixel-major
            scal = meta.tile([128, 9, KCH, 4], F16)
            idxt = meta.tile([128, 9, KCH], I32)

            with (
                tc.tile_pool(name="psc", bufs=2, space="PSUM") as psc,
                tc.tile_pool(name="pst1", bufs=2, space="PSUM") as pst1,
            ):
                # PE p-state warmup: junk matmuls keep the PE busy from t=0 so
                # the conv matmuls run at full clock once xpad lands
                warm = psc.tile([18, 512], F32, tag="po", name="warm")
                for _ in range(4):
                    nc.tensor.matmul(warm[:], junka[:, 0:18], junkb[:],
                                     start=True, stop=True)

                # Front-end in two phases: a small first phase (conv tiles
                # 0:2, chunks 0:8) so the first gathers start early, then the
                # rest in one pass. Each phase runs the full offsets path
                # (po conv -> trA -> sel -> idx) in pass A, then the mod path
                # (pm conv -> trB -> scales) in pass B, so gathers never wait
                # on the mod conv (which needs the whole image).
                Fall = meta.tile([128, 9, KCH, 2], F32)
                for tl_lo, tl_hi, k_lo, k_hi in ((0, 2, 0, 8), (2, 8, 8, 32)):
                    HK = k_hi - k_lo
                    ho = k_lo
                    # pass A: offsets conv
                    for tl in range(tl_lo, tl_hi):
                        po = psc.tile([18, 512], F32, tag="po")
                        for t in range(9):
                            dy, dx = t // 3, t % 3
                            rhs1 = bass.AP(
                                tensor=xpad[:].tensor,
                                offset=xpad[:].offset + dy * 66 + dx + tl * 8 * 66,
                                ap=[list(xpad[:].ap[0]), [66, 8], [1, 64]],
                            )
                            nc.tensor.matmul(po[:], womb[:, t, :], rhs1,
                                             start=(t == 0), stop=(t == 8))
                        nc.scalar.activation(
                            ocm[0:18, tl * 512:(tl + 1) * 512], po[:],
                            mybir.ActivationFunctionType.Identity,
                            bias=ob[:], scale=1.0)
                    for tl in range(tl_lo, tl_hi):
                        ptA = pst1.tile([128, 4, 18], F32, tag="ptA", bufs=1)
                        for k4 in range(4):
                            k = tl * 4 + k4
                            nc.tensor.matmul(
                                ptA[:, k4, :],
                                ocm[0:18, k * 128:(k + 1) * 128],
                                id16[0:18, 0:18], start=True, stop=True)
                        dstA = bass.AP(
                            tensor=opm[:].tensor,
                            offset=opm[:].offset + tl * 4 * 41,
                            ap=[list(opm[:].ap[0]), [41, 4], [1, 18]],
                        )
                        nc.vector.tensor_copy(dstA, ptA[:])
                    # pass A metadata: -> flat row idx (+ frac, kept for B)
                    for n2 in range(9):
                        a2 = n2 // 3
                        oyx = pst1.tile([128, HK, 2], F32, tag="sel")
                        for r in range(3):
                            ch = 3 * r + a2
                            rhs = bass.AP(
                                tensor=opm[:].tensor,
                                offset=opm[:].offset + ch + ho * 41,
                                ap=[list(opm[:].ap[0]), [41, HK], [9, 2]],
                            )
                            nc.tensor.matmul(oyx[:], selt[:, n2, r, :], rhs,
                                             start=(r == 0), stop=(r == 2))
                        P = meta.tile([128, HK, 2], F32, tag="P")
                        nc.vector.tensor_add(P[:], oyx[:],
                                             basyx[:, n2, ho:ho + HK, :])
                        nc.vector.tensor_scalar(P[:], P[:], 0.0, 63.0,
                                                AO.max, AO.min)
                        R0 = meta.tile([128, HK, 2], F32, tag="R0")
                        nc.vector.tensor_scalar(R0[:], P[:], -0.5, 12582912.0,
                                                AO.add, AO.add)
                        nc.vector.tensor_scalar_add(R0[:], R0[:], -12582912.0)
                        F = Fall[:, n2, ho:ho + HK, :]
                        nc.vector.tensor_sub(F, P[:], R0[:])
                        f00 = meta.tile([128, HK], F32, tag="f00")
                        nc.vector.scalar_tensor_tensor(
                            f00[:], R0[:, :, 1], 64.0, R0[:, :, 0], AO.mult, AO.add)
                        nc.vector.tensor_copy(idxt[:, n2, ho:ho + HK], f00[:])
                    # pass B: mod conv
                    for tl in range(tl_lo, tl_hi):
                        pm = psc.tile([9, 512], F32, tag="pm")
                        for t in range(9):
                            dy, dx = t // 3, t % 3
                            rhs2 = bass.AP(
                                tensor=xpad[:].tensor,
                                offset=xpad[:].offset + dx * 66 + dy + tl * 8,
                                ap=[list(xpad[:].ap[0]), [1, 8], [66, 64]],
                            )
                            nc.tensor.matmul(pm[:], wmtb[:, t, :], rhs2,
                                             start=(t == 0), stop=(t == 8))
                        nc.scalar.activation(
                            ocm[32:41, tl * 512:(tl + 1) * 512], pm[:],
                            mybir.ActivationFunctionType.Sigmoid,
                            bias=mb[:], scale=1.0)
                    for tl in range(tl_lo, tl_hi):
                        ptB = pst1.tile([128, 4, 9], F32, tag="ptB", bufs=1)
                        for k4 in range(4):
                            k = tl * 4 + k4
                            nc.tensor.matmul(
                                ptB[:, k4, :],
                                ocm[32:41, k * 128:(k + 1) * 128],
                                id16[32:41, 32:41], start=True, stop=True)
                        dstB = bass.AP(
                            tensor=opm[:].tensor,
                            offset=opm[:].offset + tl * 4 * 41 + 32,
                            ap=[list(opm[:].ap[0]), [41, 4], [1, 9]],
                        )
                        nc.vector.tensor_copy(dstB, ptB[:])
                    # pass B metadata: bilinear*modulation scales (fp16)
                    for n2 in range(9):
                        F = Fall[:, n2, ho:ho + HK, :]
                        mrow = opm[:, ho:ho + HK, 32 + n2]
                        v1 = meta.tile([128, HK], F32, tag="v1")
                        v0 = meta.tile([128, HK], F32, tag="v0")
                        sc4 = meta.tile([128, 4, HK], F32, tag="sc4")
                        nc.vector.tensor_mul(v1[:], mrow, F[:, :, 1])
                        nc.vector.tensor_sub(v0[:], mrow, v1[:])
                        nc.vector.tensor_mul(sc4[:, 1, :], v0[:], F[:, :, 0])
                        nc.vector.tensor_sub(sc4[:, 0, :], v0[:], sc4[:, 1, :])
                        nc.vector.tensor_mul(sc4[:, 3, :], v1[:], F[:, :, 0])
                        nc.vector.tensor_sub(sc4[:, 2, :], v1[:], sc4[:, 3, :])
                        # convert to fp16 [k, corner]-interleaved in one copy
                        csrc = bass.AP(
                            tensor=sc4[:].tensor, offset=sc4[:].offset,
                            ap=[list(sc4[:].ap[0]), [1, HK], [HK, 4]],
                        )
                        cdst = bass.AP(
                            tensor=scal[:].tensor,
                            offset=scal[:].offset + n2 * (KCH * 4) + ho * 4,
                            ap=[list(scal[:].ap[0]), [4, HK], [1, 4]],
                        )
                        nc.vector.tensor_copy(cdst, csrc)

            with (
                tc.tile_pool(name="pst", bufs=3, space="PSUM") as pst,
                tc.tile_pool(name="psm", bufs=1, space="PSUM") as psm,
            ):
                # ---- per spatial-quarter: gather + scale + reduce-transpose;
                #      main-conv matmuls interleave per n2 (PSUM accumulates
                #      while later n2 groups are still gathering)
                for sq in range(4):
                    vc = vbuf.tile([C, 9, 1024], F16, tag="vc")
                    accq = {}
                    for hf in range(2):
                        for tl2 in range(2):
                            accq[hf, tl2] = psm.tile(
                                [128, 512], F32, tag=f"mm{hf}{tl2}",
                                name=f"acc{hf}{tl2}")
                    for n2 in range(9):
                        g = gbuf.tile([128, 8, 512], F16, tag="g")
                        for kk in range(8):
                            k = sq * 8 + kk
                            dstg = bass.AP(
                                tensor=g[:].tensor,
                                offset=g[:].offset + kk * 512,
                                ap=[list(g[:].ap[0]), [1, 512]],
                            )
                            nc.gpsimd.indirect_dma_start(
                                out=dstg, out_offset=None,
                                in_=d["ptab"][:],
                                in_offset=bass.IndirectOffsetOnAxis(
                                    ap=idxt[:, n2, k:k + 1], axis=0),
                            )
                        h = hbuf.tile([128, 8, 128, 2], F16, tag="h")
                        for q in range(2):
                            gv = bass.AP(
                                tensor=g[:].tensor,
                                offset=g[:].offset + q * 2048,
                                ap=[list(g[:].ap[0]), [512, 4], [4, 128], [1, 4]],
                            )
                            sv = bass.AP(
                                tensor=scal[:].tensor,
                                offset=(scal[:].offset + n2 * (KCH * 4)
                                        + sq * 32 + q * 16),
                                ap=[list(scal[:].ap[0]), [4, 4], [0, 128], [1, 4]],
                            )
                            nc.vector.tensor_mul(gv, gv, sv)
                            ha = bass.AP(
                                tensor=g[:].tensor,
                                offset=g[:].offset + q * 2048,
                                ap=[list(g[:].ap[0]), [512, 4], [4, 128], [1, 2]],
                            )
                            hb = bass.AP(
                                tensor=g[:].tensor,
                                offset=g[:].offset + q * 2048 + 2,
                                ap=[list(g[:].ap[0]), [512, 4], [4, 128], [1, 2]],
                            )
                            hd = bass.AP(
                                tensor=h[:].tensor,
                                offset=h[:].offset + q * 1024,
                                ap=[list(h[:].ap[0]), [256, 4], [2, 128], [1, 2]],
                            )
                            nc.vector.tensor_add(hd, ha, hb)
                            acc = pst.tile([128, 512], F32, tag="tr")
                            for kk4 in range(4):
                                kk = q * 4 + kk4
                                for j in range(2):
                                    lhsT = bass.AP(
                                        tensor=h[:].tensor,
                                        offset=h[:].offset + kk * 256 + j,
                                        ap=[list(h[:].ap[0]), [2, 128]],
                                    )
                                    nc.tensor.matmul(
                                        acc[:, kk4 * 128:(kk4 + 1) * 128],
                                        lhsT, id16[:],
                                        start=(j == 0), stop=(j == 1))
                            nc.scalar.copy(vc[:, n2, q * 512:(q + 1) * 512], acc[:])
                            for hf in range(2):
                                nc.tensor.matmul(
                                    accq[hf, q][:], w2[:, n2, hf, :],
                                    vc[:, n2, q * 512:(q + 1) * 512],
                                    start=(n2 == 0), stop=(n2 == 8))

                    # store raw blocks (contiguous; host unscrambles)
                    for hf in range(2):
                        for q in range(2):
                            outq = obuf.tile([128, 512], F32, tag="oq",
                                             name="outq")
                            nc.scalar.copy(outq[:], accq[hf, q][:])
                            nc.sync.dma_start(
                                d["outr"][sq * 4 + hf * 2 + q], outq[:])

    nc.compile()
    _CACHE["nc"] = nc
    return nc


def _host_inputs(b_x, offset_w, offset_b, mod_w, mod_b, conv_w):
    hc = _build_host_constants()
    img = b_x.astype(np.float32)
    womb = np.zeros((C, 9, 18), np.float16)
    wmtb = np.zeros((C, 9, 9), np.float16)
    for t in range(9):
        dy, dx = t // 3, t % 3
        womb[:, t, :] = offset_w[:, :, dy, dx].T
        wmtb[:, 3 * dx + dy, :] = mod_w[:, :, dy, dx].T
    w2 = np.zeros((C, 9, 2, 128), np.float16)
    for n2 in range(9):
        a2, e2 = n2 // 3, n2 % 3
        for hf in range(2):
            w2[:, n2, hf, :] = conv_w[128 * hf:128 * (hf + 1), :, a2, e2].T
    return {
        "xpad": _pad66(img),
        "ptab": _patch_table(img),
        "womb": womb,
        "wmtb": wmtb,
        "ob": offset_b.reshape(18, 1).astype(np.float32),
        "mb": mod_b.reshape(9, 1).astype(np.float32),
        "selt": hc["sel"].reshape(128, 9 * 3 * 128),
        "basyx": hc["basyx"].reshape(128, 9 * KCH * 2),
        "w2": w2.reshape(C, 9 * 2 * 128),
        "id16": hc["ident16"],
    }


def kernel(x, offset_w, offset_b, mod_w, mod_b, conv_w):
    nc = _build_program()
    in_maps = [
        _host_inputs(x[b], offset_w, offset_b, mod_w, mod_b, conv_w)
        for b in range(B)
    ]
    res = run_bass_kernel_spmd(nc, in_maps, core_ids=list(range(B)))
    out = np.empty((B, OUT, H, W), np.float32)
    for b in range(B):
        # outr[sq*4 + hf*2 + q] = [128 o, 512 pi2'] with
        # pi2' = (2sq+q)*512 + q2, j2 = 8*(2sq+q) + q2//64, i2 = q2%64
        outr = res.results[b]["outr"].reshape(4, 2, 2, 128, 8, 64)
        for sq in range(4):
            for hf in range(2):
                for q in range(2):
                    j2 = 16 * sq + 8 * q
                    out[b, 128 * hf:128 * (hf + 1), :, j2:j2 + 8] = (
                        outr[sq, hf, q].transpose(0, 2, 1))
    return out


if __name__ == "__main__":
    rng = np.random.default_rng(0)
    ins = {
        "x": rng.standard_normal((B, C, H, W), dtype=np.float32),
        "offset_w": (rng.standard_normal((18, C, 3, 3)) / 34).astype(np.float32),
        "offset_b": (rng.standard_normal(18) * 0.01).astype(np.float32),
        "mod_w": (rng.standard_normal((9, C, 3, 3)) / 34).astype(np.float32),
        "mod_b": (rng.standard_normal(9) * 0.01).astype(np.float32),
        "conv_w": (rng.standard_normal((OUT, C, 3, 3)) / 34).astype(np.float32),
    }
    o = kernel(**ins)
    print("out", o.shape, o.dtype, np.abs(o).max())


# revision 46
# speedup vs baseline: 1.0171x; 1.0050x over previous
# Deformable-conv (DCNv2-style, scrambled-reshape variant) Trainium2 Bass kernel.
# Data-parallel over batch: 8 samples -> 8 NeuronCores.
#
# Per-core pipeline (layouts derived + validated against the reference):
#   1. offset conv (18ch) + modulation conv (9ch) in ONE fp16 pass over padded x.
#      The mod conv runs on the transposed image via a transposed access
#      pattern on the same xpad tile (no second image needed); outputs land in
#      one [27, 4096] fp16 tile (rows 0:18 offsets, 18:27 sigmoid(mod)).
#   2. PE "transposes" (regular fp16 matmuls vs identity) to pixel-major
#      [128 pix, 32 chunk, 27].
#   3. Per kernel-point n2: 3 host-constant selection matmuls pick the
#      (source-pixel, source-channel) pair per partition; pointwise metadata
#      (DVE) produces a flat 2x2-patch row index + 4 bilinear*modulation
#      scales (fp16, corner-innermost).
#   4. Indirect-DMA gathers from a host-built patch table whose rows are
#      channel-outer/corner-inner (row f = [c0:4 corners, c1:4 corners, ...]),
#      so the scale multiply has packed fp16 innermost dims on every operand
#      (DVE 2x mode). One [128,1]-offset gather per (n2, chunk).
#   5. One DVE mul (scales) + one DVE pair-add (4 corners -> 2), then the
#      final corner reduction + transpose to channel-major happen on the PE:
#      2 PSUM-accumulated matmuls per 128-pixel chunk against identity.
#   6. Main conv = 9 accumulated fp16 matmuls per 512-pixel block; PSUM
#      copies write through a transposed AP into a full-row [128, 4096]
#      staging tile, stored with one contiguous DMA per 128-channel half.
import sys

import numpy as np

sys.path.insert(0, "/opt/trn_rl_repo")

import concourse.bass as bass
import concourse.bacc as bacc
import concourse.mybir as mybir
from concourse import tile
from concourse.bass_utils import run_bass_kernel_spmd

F32 = mybir.dt.float32
F16 = mybir.dt.float16
I32 = mybir.dt.int32

B, C, H, W = 8, 128, 64, 64
OUT = 256
PIX = H * W            # 4096
KCH = 32               # pixel-major chunks (4096 / 128)
TROWS = 4224           # patch table rows (4096 + pad for f+65 reads)

_CACHE = {}


def _build_host_constants():
    if "sel" in _CACHE:
        return _CACHE
    p2 = np.arange(128)
    k2 = np.arange(KCH)
    sel = np.zeros((128, 9, 3, 128), np.float16)   # [p_src, n2, r, p2]
    basey = np.zeros((128, 9, KCH), np.float32)    # [p, n2, k]
    basex = np.zeros((128, 9, KCH), np.float32)
    for n2 in range(9):
        a2, e2 = n2 // 3, n2 % 3
        i2 = p2 % 64
        r = (i2 + e2) % 3
        n = 3 * r + a2                       # source kernel point per partition
        J = (64 * e2 + i2) // 3              # source col j per partition
        c_src = 64 * (p2 // 64) + J          # source partition in pixel-major
        for rr in range(3):
            m = r == rr
            sel[c_src[m], n2, rr, p2[m]] = 1.0
        a = n // 3
        e = n % 3
        # y_u = i + a + o_y ; i = j2 = 2*k2 + p2//64
        basey[:, n2, :] = (2 * k2[None, :] + (p2 // 64)[:, None]) + a[:, None]
        basex[:, n2, :] = (J + e)[:, None] * np.ones((1, KCH), np.float32)
    _CACHE["sel"] = sel
    _CACHE["basyx"] = np.ascontiguousarray(np.stack([basey, basex], axis=-1))
    _CACHE["ident16"] = np.eye(128, dtype=np.float16)
    return _CACHE


def _pad66(img):  # [C,64,64] -> [C, 66*66] zero-padded fp16
    p = np.zeros((C, 66, 66), np.float16)
    p[:, 1:65, 1:65] = img
    return p.reshape(C, 66 * 66)


def _patch_table(img):  # [C,64,64] f32 -> [TROWS, 512] fp16, channel-outer rows
    flat = np.zeros((C, TROWS + 65), np.float16)
    flat[:, :PIX] = img.reshape(C, PIX).astype(np.float16)
    f = np.arange(TROWS)
    tab = np.stack(
        [flat[:, f], flat[:, f + 1], flat[:, f + 64], flat[:, f + 65]], axis=-1
    )  # [C, TROWS, 4]
    return np.ascontiguousarray(tab.transpose(1, 0, 2)).reshape(TROWS, 512)


def _build_program():
    if "nc" in _CACHE:
        return _CACHE["nc"]
    nc = bacc.Bacc()
    d = {}
    d["xpad"] = nc.dram_tensor("xpad", [C, 66 * 66], F16, kind="ExternalInput")
    d["ptab"] = nc.dram_tensor("ptab", [TROWS, 512], F16, kind="ExternalInput")
    d["womb"] = nc.dram_tensor("womb", [C, 9, 18], F16, kind="ExternalInput")
    d["wmtb"] = nc.dram_tensor("wmtb", [C, 9, 9], F16, kind="ExternalInput")
    d["ob"] = nc.dram_tensor("ob", [18, 1], F32, kind="ExternalInput")
    d["mb"] = nc.dram_tensor("mb", [9, 1], F32, kind="ExternalInput")
    d["selt"] = nc.dram_tensor("selt", [128, 9 * 3 * 128], F16, kind="ExternalInput")
    d["basyx"] = nc.dram_tensor("basyx", [128, 9 * KCH * 2], F32,
                                kind="ExternalInput")
    d["w2"] = nc.dram_tensor("w2", [C, 9 * 2 * 128], F16, kind="ExternalInput")
    d["id16"] = nc.dram_tensor("id16", [128, 128], F16, kind="ExternalInput")
    # raw main-conv PSUM blocks [sq*4 + hf*2 + q] = [128 out-ch, 512 pi2'];
    # the fixed pi2'->pixel permutation happens on host during unshard
    d["outr"] = nc.dram_tensor("outr", [16, 128, 512], F32, kind="ExternalOutput")

    AO = mybir.AluOpType

    with tile.TileContext(nc) as tc:
        with (
            tc.tile_pool(name="imgs", bufs=1) as imgs,
            tc.tile_pool(name="wts", bufs=1) as wts,
            tc.tile_pool(name="meta", bufs=1) as meta,
            tc.tile_pool(name="gbuf", bufs=8) as gbuf,
            tc.tile_pool(name="hbuf", bufs=2) as hbuf,
            tc.tile_pool(name="vbuf", bufs=2) as vbuf,
            tc.tile_pool(name="obuf", bufs=4) as obuf,
        ):
            # ---- load image + weights + constants (single DMAs each; order =
            #      first-use order so the conv pipeline starts ASAP)
            # loads ordered by first use on the critical path: the offsets
            # conv (xpad rows 0:10 + womb) unblocks first, big slices later
            xpad = imgs.tile([C, 66 * 66], F16)
            nc.sync.dma_start(xpad[:, 0:1188], d["xpad"][:, 0:1188])
            womb = wts.tile([C, 9, 18], F16)
            nc.sync.dma_start(womb[:], d["womb"][:])
            ob = wts.tile([18, 1], F32)
            nc.sync.dma_start(ob[:], d["ob"][:])
            id16 = wts.tile([128, 128], F16)
            nc.sync.dma_start(id16[:], d["id16"][:])
            selt = wts.tile([128, 9, 3, 128], F16)
            nc.sync.dma_start(selt[:], d["selt"][:])
            basyx = wts.tile([128, 9, KCH, 2], F32)
            nc.sync.dma_start(basyx[:], d["basyx"][:])
            nc.sync.dma_start(xpad[:, 1188:66 * 34], d["xpad"][:, 1188:66 * 34])
            wmtb = wts.tile([C, 9, 9], F16)
            nc.sync.dma_start(wmtb[:], d["wmtb"][:])
            mb = wts.tile([9, 1], F32)
            nc.sync.dma_start(mb[:], d["mb"][:])
            nc.sync.dma_start(xpad[:, 66 * 34:], d["xpad"][:, 66 * 34:])
            w2 = wts.tile([C, 9, 2, 128], F16)
            nc.sync.dma_start(w2[:], d["w2"][:])
            # junk tiles for PE p-state warmup + Act table preload (values
            # never consumed)
            junka = wts.tile([128, 128], F16)
            junkb = wts.tile([128, 512], F16)
            junkc = wts.tile([18, 4], F16)
            nc.vector.memset(junka[:], 0.0)
            nc.vector.memset(junkb[:], 0.0)
            nc.scalar.activation(junkc[:], junka[0:18, 0:4],
                                 mybir.ActivationFunctionType.Identity,
                                 bias=0.0, scale=1.0)
            nc.scalar.activation(junkc[:], junka[0:18, 0:4],
                                 mybir.ActivationFunctionType.Sigmoid,
                                 bias=0.0, scale=1.0)

            # rows 0:18 offsets, 32:41 mod (engine outputs need 32-aligned
            # partition starts; rows 18:32 stay uninitialized and are never
            # read -- the transposes contract only 0:18 / 32:41)
            ocm = meta.tile([41, PIX], F16)
            opm = meta.tile([128, KCH, 41], F16)   # pixel-major
            scal = meta.tile([128, 9, KCH, 4], F16)
            idxt = meta.tile([128, 9, KCH], I32)

            with (
                tc.tile_pool(name="psc", bufs=2, space="PSUM") as psc,
                tc.tile_pool(name="pst1", bufs=2, space="PSUM") as pst1,
            ):
                # PE p-state warmup: junk matmuls keep the PE busy from t=0 so
                # the conv matmuls run at full clock once xpad lands
                warm = psc.tile([18, 512], F32, tag="po", name="warm")
                for _ in range(4):
                    nc.tensor.matmul(warm[:], junka[:, 0:18], junkb[:],
                                     start=True, stop=True)

                # Front-end in two phases: a small first phase (conv tiles
                # 0:2, chunks 0:8) so the first gathers start early, then the
                # rest in one pass. Each phase runs the full offsets path
                # (po conv -> trA -> sel -> idx) in pass A, then the mod path
                # (pm conv -> trB -> scales) in pass B, so gathers never wait
                # on the mod conv (which needs the whole image).
                Fall = meta.tile([128, 9, KCH, 2], F32)
                for tl_lo, tl_hi, k_lo, k_hi in ((0, 2, 0, 8), (2, 8, 8, 32)):
                    HK = k_hi - k_lo
                    ho = k_lo
                    # pass A: offsets conv
                    for tl in range(tl_lo, tl_hi):
                        po = psc.tile([18, 512], F32, tag="po")
                        for t in range(9):
                            dy, dx = t // 3, t % 3
                            rhs1 = bass.AP(
                                tensor=xpad[:].tensor,
                                offset=xpad[:].offset + dy * 66 + dx + tl * 8 * 66,
                                ap=[list(xpad[:].ap[0]), [66, 8], [1, 64]],
                            )
                            nc.tensor.matmul(po[:], womb[:, t, :], rhs1,
                                             start=(t == 0), stop=(t == 8))
                        nc.scalar.activation(
                            ocm[0:18, tl * 512:(tl + 1) * 512], po[:],
                            mybir.ActivationFunctionType.Identity,
                            bias=ob[:], scale=1.0)
                    for tl in range(tl_lo, tl_hi):
                        ptA = pst1.tile([128, 4, 18], F32, tag="ptA", bufs=1)
                        for k4 in range(4):
                            k = tl * 4 + k4
                            nc.tensor.matmul(
                                ptA[:, k4, :],
                                ocm[0:18, k * 128:(k + 1) * 128],
                                id16[0:18, 0:18], start=True, stop=True)
                        dstA = bass.AP(
                            tensor=opm[:].tensor,
                            offset=opm[:].offset + tl * 4 * 41,
                            ap=[list(opm[:].ap[0]), [41, 4], [1, 18]],
                        )
                        nc.vector.tensor_copy(dstA, ptA[:])
                    # pass A metadata: -> flat row idx (+ frac, kept for B)
                    for n2 in range(9):
                        a2 = n2 // 3
                        oyx = pst1.tile([128, HK, 2], F32, tag="sel")
                        for r in range(3):
                            ch = 3 * r + a2
                            rhs = bass.AP(
                                tensor=opm[:].tensor,
                                offset=opm[:].offset + ch + ho * 41,
                                ap=[list(opm[:].ap[0]), [41, HK], [9, 2]],
                            )
                            nc.tensor.matmul(oyx[:], selt[:, n2, r, :], rhs,
                                             start=(r == 0), stop=(r == 2))
                        P = meta.tile([128, HK, 2], F32, tag="P")
                        nc.vector.tensor_add(P[:], oyx[:],
                                             basyx[:, n2, ho:ho + HK, :])
                        nc.vector.tensor_scalar(P[:], P[:], 0.0, 63.0,
                                                AO.max, AO.min)
                        R0 = meta.tile([128, HK, 2], F32, tag="R0")
                        nc.vector.tensor_scalar(R0[:], P[:], -0.5, 12582912.0,
                                                AO.add, AO.add)
                        nc.vector.tensor_scalar_add(R0[:], R0[:], -12582912.0)
                        F = Fall[:, n2, ho:ho + HK, :]
                        nc.vector.tensor_sub(F, P[:], R0[:])
                        f00 = meta.tile([128, HK], F32, tag="f00")
                        nc.vector.scalar_tensor_tensor(
                            f00[:], R0[:, :, 1], 64.0, R0[:, :, 0], AO.mult, AO.add)
                        nc.vector.tensor_copy(idxt[:, n2, ho:ho + HK], f00[:])
                    # pass B: mod conv
                    for tl in range(tl_lo, tl_hi):
                        pm = psc.tile([9, 512], F32, tag="pm")
                        for t in range(9):
                            dy, dx = t // 3, t % 3
                            rhs2 = bass.AP(
                                tensor=xpad[:].tensor,
                                offset=xpad[:].offset + dx * 66 + dy + tl * 8,
                                ap=[list(xpad[:].ap[0]), [1, 8], [66, 64]],
                            )
                            nc.tensor.matmul(pm[:], wmtb[:, t, :], rhs2,
                                             start=(t == 0), stop=(t == 8))
                        nc.scalar.activation(
                            ocm[32:41, tl * 512:(tl + 1) * 512], pm[:],
                            mybir.ActivationFunctionType.Sigmoid,
                            bias=mb[:], scale=1.0)
                    for tl in range(tl_lo, tl_hi):
                        ptB = pst1.tile([128, 4, 9], F32, tag="ptB", bufs=1)
                        for k4 in range(4):
                            k = tl * 4 + k4
                            nc.tensor.matmul(
                                ptB[:, k4, :],
                                ocm[32:41, k * 128:(k + 1) * 128],
                                id16[32:41, 32:41], start=True, stop=True)
                        dstB = bass.AP(
                            tensor=opm[:].tensor,
                            offset=opm[:].offset + tl * 4 * 41 + 32,
                            ap=[list(opm[:].ap[0]), [41, 4], [1, 9]],
                        )
                        nc.vector.tensor_copy(dstB, ptB[:])
                    # pass B metadata: bilinear*modulation scales (fp16)
                    for n2 in range(9):
                        F = Fall[:, n2, ho:ho + HK, :]
                        mrow = opm[:, ho:ho + HK, 32 + n2]
                        v1 = meta.tile([128, HK], F32, tag="v1")
                        v0 = meta.tile([128, HK], F32, tag="v0")
                        sc4 = meta.tile([128, 4, HK], F32, tag="sc4")
                        nc.vector.tensor_mul(v1[:], mrow, F[:, :, 1])
                        nc.vector.tensor_sub(v0[:], mrow, v1[:])
                        nc.vector.tensor_mul(sc4[:, 1, :], v0[:], F[:, :, 0])
                        nc.vector.tensor_sub(sc4[:, 0, :], v0[:], sc4[:, 1, :])
                        nc.vector.tensor_mul(sc4[:, 3, :], v1[:], F[:, :, 0])
                        nc.vector.tensor_sub(sc4[:, 2, :], v1[:], sc4[:, 3, :])
                        # convert to fp16 [k, corner]-interleaved in one copy
                        csrc = bass.AP(
                            tensor=sc4[:].tensor, offset=sc4[:].offset,
                            ap=[list(sc4[:].ap[0]), [1, HK], [HK, 4]],
                        )
                        cdst = bass.AP(
                            tensor=scal[:].tensor,
                            offset=scal[:].offset + n2 * (KCH * 4) + ho * 4,
                            ap=[list(scal[:].ap[0]), [4, HK], [1, 4]],
                        )
                        nc.vector.tensor_copy(cdst, csrc)

            with (
                tc.tile_pool(name="pst", bufs=3, space="PSUM") as pst,
                tc.tile_pool(name="psm", bufs=1, space="PSUM") as psm,
            ):
                # ---- per spatial-quarter: gather + scale + reduce-transpose;
                #      main-conv matmuls interleave per n2 (PSUM accumulates
                #      while later n2 groups are still gathering)
                for sq in range(4):
                    vc = vbuf.tile([C, 9, 1024], F16, tag="vc")
                    accq = {}
                    for hf in range(2):
                        for tl2 in range(2):
                            accq[hf, tl2] = psm.tile(
                                [128, 512], F32, tag=f"mm{hf}{tl2}",
                                name=f"acc{hf}{tl2}")
                    for n2 in range(9):
                        g = gbuf.tile([128, 8, 512], F16, tag="g")
                        for kk in range(8):
                            k = sq * 8 + kk
                            dstg = bass.AP(
                                tensor=g[:].tensor,
                                offset=g[:].offset + kk * 512,
                                ap=[list(g[:].ap[0]), [1, 512]],
                            )
                            nc.gpsimd.indirect_dma_start(
                                out=dstg, out_offset=None,
                                in_=d["ptab"][:],
                                in_offset=bass.IndirectOffsetOnAxis(
                                    ap=idxt[:, n2, k:k + 1], axis=0),
                            )
                        h = hbuf.tile([128, 8, 128, 2], F16, tag="h")
                        for q in range(2):
                            gv = bass.AP(
                                tensor=g[:].tensor,
                                offset=g[:].offset + q * 2048,
                                ap=[list(g[:].ap[0]), [512, 4], [4, 128], [1, 4]],
                            )
                            sv = bass.AP(
                                tensor=scal[:].tensor,
                                offset=(scal[:].offset + n2 * (KCH * 4)
                                        + sq * 32 + q * 16),
                                ap=[list(scal[:].ap[0]), [4, 4], [0, 128], [1, 4]],
                            )
                            nc.vector.tensor_mul(gv, gv, sv)
                            ha = bass.AP(
                                tensor=g[:].tensor,
                                offset=g[:].offset + q * 2048,
                                ap=[list(g[:].ap[0]), [512, 4], [4, 128], [1, 2]],
                            )
                            hb = bass.AP(
                                tensor=g[:].tensor,
                                offset=g[:].offset + q * 2048 + 2,
                                ap=[list(g[:].ap[0]), [512, 4], [4, 128], [1, 2]],
                            )
                            hd = bass.AP(
                                tensor=h[:].tensor,
                                offset=h[:].offset + q * 1024,
                                ap=[list(h[:].ap[0]), [256, 4], [2, 128], [1, 2]],
                            )
                            nc.vector.tensor_add(hd, ha, hb)
                            acc = pst.tile([128, 512], F32, tag="tr")
                            for kk4 in range(4):
                                kk = q * 4 + kk4
                                for j in range(2):
                                    lhsT = bass.AP(
                                        tensor=h[:].tensor,
                                        offset=h[:].offset + kk * 256 + j,
                                        ap=[list(h[:].ap[0]), [2, 128]],
                                    )
                                    nc.tensor.matmul(
                                        acc[:, kk4 * 128:(kk4 + 1) * 128],
                                        lhsT, id16[:],
                                        start=(j == 0), stop=(j == 1))
                            nc.scalar.copy(vc[:, n2, q * 512:(q + 1) * 512], acc[:])
                            for hf in range(2):
                                nc.tensor.matmul(
                                    accq[hf, q][:], w2[:, n2, hf, :],
                                    vc[:, n2, q * 512:(q + 1) * 512],
                                    start=(n2 == 0), stop=(n2 == 8))

                    # store raw blocks (contiguous; host unscrambles)
                    for hf in range(2):
                        for q in range(2):
                            outq = obuf.tile([128, 512], F32, tag="oq",
                                             name="outq")
                            nc.scalar.copy(outq[:], accq[hf, q][:])
                            nc.sync.dma_start(
                                d["outr"][sq * 4 + hf * 2 + q], outq[:])

    nc.compile()
    _CACHE["nc"] = nc
    return nc


def _host_inputs(b_x, offset_w, offset_b, mod_w, mod_b, conv_w):
    hc = _build_host_constants()
    img = b_x.astype(np.float32)
    womb = np.zeros((C, 9, 18), np.float16)
    wmtb = np.zeros((C, 9, 9), np.float16)
    for t in range(9):
        dy, dx = t // 3, t % 3
        womb[:, t, :] = offset_w[:, :, dy, dx].T
        wmtb[:, 3 * dx + dy, :] = mod_w[:, :, dy, dx].T
    w2 = np.zeros((C, 9, 2, 128), np.float16)
    for n2 in range(9):
        a2, e2 = n2 // 3, n2 % 3
        for hf in range(2):
            w2[:, n2, hf, :] = conv_w[128 * hf:128 * (hf + 1), :, a2, e2].T
    return {
        "xpad": _pad66(img),
        "ptab": _patch_table(img),
        "womb": womb,
        "wmtb": wmtb,
        "ob": offset_b.reshape(18, 1).astype(np.float32),
        "mb": mod_b.reshape(9, 1).astype(np.float32),
        "selt": hc["sel"].reshape(128, 9 * 3 * 128),
        "basyx": hc["basyx"].reshape(128, 9 * KCH * 2),
        "w2": w2.reshape(C, 9 * 2 * 128),
        "id16": hc["ident16"],
    }


def kernel(x, offset_w, offset_b, mod_w, mod_b, conv_w):
    nc = _build_program()
    in_maps = [
        _host_inputs(x[b], offset_w, offset_b, mod_w, mod_b, conv_w)
        for b in range(B)
    ]
    res = run_bass_kernel_spmd(nc, in_maps, core_ids=list(range(B)))
    out = np.empty((B, OUT, H, W), np.float32)
    for b in range(B):
        # outr[sq*4 + hf*2 + q] = [128 o, 512 pi2'] with
        # pi2' = (2sq+q)*512 + q2, j2 = 8*(2sq+q) + q2//64, i2 = q2%64
        outr = res.results[b]["outr"].reshape(4, 2, 2, 128, 8, 64)
        for sq in range(4):
            for hf in range(2):
                for q in range(2):
                    j2 = 16 * sq + 8 * q
                    out[b, 128 * hf:128 * (hf + 1), :, j2:j2 + 8] = (
                        outr[sq, hf, q].transpose(0, 2, 1))
    return out


if __name__ == "__main__":
    rng = np.random.default_rng(0)
    ins = {
        "x": rng.standard_normal((B, C, H, W), dtype=np.float32),
        "offset_w": (rng.standard_normal((18, C, 3, 3)) / 34).astype(np.float32),
        "offset_b": (rng.standard_normal(18) * 0.01).astype(np.float32),
        "mod_w": (rng.standard_normal((9, C, 3, 3)) / 34).astype(np.float32),
        "mod_b": (rng.standard_normal(9) * 0.01).astype(np.float32),
        "conv_w": (rng.standard_normal((OUT, C, 3, 3)) / 34).astype(np.float32),
    }
    o = kernel(**ins)
    print("out", o.shape, o.dtype, np.abs(o).max())


# revision 48
# speedup vs baseline: 1.0187x; 1.0016x over previous
# Deformable-conv (DCNv2-style, scrambled-reshape variant) Trainium2 Bass kernel.
# Data-parallel over batch: 8 samples -> 8 NeuronCores.
#
# Per-core pipeline (layouts derived + validated against the reference):
#   1. offset conv (18ch) + modulation conv (9ch) in ONE fp16 pass over padded x.
#      The mod conv runs on the transposed image via a transposed access
#      pattern on the same xpad tile (no second image needed); outputs land in
#      one [27, 4096] fp16 tile (rows 0:18 offsets, 18:27 sigmoid(mod)).
#   2. PE "transposes" (regular fp16 matmuls vs identity) to pixel-major
#      [128 pix, 32 chunk, 27].
#   3. Per kernel-point n2: 3 host-constant selection matmuls pick the
#      (source-pixel, source-channel) pair per partition; pointwise metadata
#      (DVE) produces a flat 2x2-patch row index + 4 bilinear*modulation
#      scales (fp16, corner-innermost).
#   4. Indirect-DMA gathers from a host-built patch table whose rows are
#      channel-outer/corner-inner (row f = [c0:4 corners, c1:4 corners, ...]),
#      so the scale multiply has packed fp16 innermost dims on every operand
#      (DVE 2x mode). One [128,1]-offset gather per (n2, chunk).
#   5. One DVE mul (scales) + one DVE pair-add (4 corners -> 2), then the
#      final corner reduction + transpose to channel-major happen on the PE:
#      2 PSUM-accumulated matmuls per 128-pixel chunk against identity.
#   6. Main conv = 9 accumulated fp16 matmuls per 512-pixel block; PSUM
#      copies write through a transposed AP into a full-row [128, 4096]
#      staging tile, stored with one contiguous DMA per 128-channel half.
import sys

import numpy as np

sys.path.insert(0, "/opt/trn_rl_repo")

import concourse.bass as bass
import concourse.bacc as bacc
import concourse.mybir as mybir
from concourse import tile
from concourse.bass_utils import run_bass_kernel_spmd

F32 = mybir.dt.float32
F16 = mybir.dt.float16
I32 = mybir.dt.int32

B, C, H, W = 8, 128, 64, 64
OUT = 256
PIX = H * W            # 4096
KCH = 32               # pixel-major chunks (4096 / 128)
TROWS = 4224           # patch table rows (4096 + pad for f+65 reads)

_CACHE = {}


def _build_host_constants():
    if "sel" in _CACHE:
        return _CACHE
    p2 = np.arange(128)
    k2 = np.arange(KCH)
    sel = np.zeros((128, 9, 3, 128), np.float16)   # [p_src, n2, r, p2]
    basey = np.zeros((128, 9, KCH), np.float32)    # [p, n2, k]
    basex = np.zeros((128, 9, KCH), np.float32)
    for n2 in range(9):
        a2, e2 = n2 // 3, n2 % 3
        i2 = p2 % 64
        r = (i2 + e2) % 3
        n = 3 * r + a2                       # source kernel point per partition
        J = (64 * e2 + i2) // 3              # source col j per partition
        c_src = 64 * (p2 // 64) + J          # source partition in pixel-major
        for rr in range(3):
            m = r == rr
            sel[c_src[m], n2, rr, p2[m]] = 1.0
        a = n // 3
        e = n % 3
        # y_u = i + a + o_y ; i = j2 = 2*k2 + p2//64
        basey[:, n2, :] = (2 * k2[None, :] + (p2 // 64)[:, None]) + a[:, None]
        basex[:, n2, :] = (J + e)[:, None] * np.ones((1, KCH), np.float32)
    _CACHE["sel"] = sel
    _CACHE["basyx"] = np.ascontiguousarray(np.stack([basey, basex], axis=-1))
    _CACHE["ident16"] = np.eye(128, dtype=np.float16)
    return _CACHE


def _pad66(img):  # [C,64,64] -> [C, 66*66] zero-padded fp16
    p = np.zeros((C, 66, 66), np.float16)
    p[:, 1:65, 1:65] = img
    return p.reshape(C, 66 * 66)


def _patch_table(img):  # [C,64,64] f32 -> [TROWS, 512] fp16, channel-outer rows
    flat = np.zeros((C, TROWS + 65), np.float16)
    flat[:, :PIX] = img.reshape(C, PIX).astype(np.float16)
    f = np.arange(TROWS)
    tab = np.stack(
        [flat[:, f], flat[:, f + 1], flat[:, f + 64], flat[:, f + 65]], axis=-1
    )  # [C, TROWS, 4]
    return np.ascontiguousarray(tab.transpose(1, 0, 2)).reshape(TROWS, 512)


def _build_program():
    if "nc" in _CACHE:
        return _CACHE["nc"]
    nc = bacc.Bacc()
    d = {}
    d["xpad"] = nc.dram_tensor("xpad", [C, 66 * 66], F16, kind="ExternalInput")
    d["ptab"] = nc.dram_tensor("ptab", [TROWS, 512], F16, kind="ExternalInput")
    d["womb"] = nc.dram_tensor("womb", [C, 9, 18], F16, kind="ExternalInput")
    d["wmtb"] = nc.dram_tensor("wmtb", [C, 9, 9], F16, kind="ExternalInput")
    d["ob"] = nc.dram_tensor("ob", [18, 1], F32, kind="ExternalInput")
    d["mb"] = nc.dram_tensor("mb", [9, 1], F32, kind="ExternalInput")
    d["selt"] = nc.dram_tensor("selt", [128, 9 * 3 * 128], F16, kind="ExternalInput")
    d["basyx"] = nc.dram_tensor("basyx", [128, 9 * KCH * 2], F32,
                                kind="ExternalInput")
    d["w2"] = nc.dram_tensor("w2", [C, 9 * 2 * 128], F16, kind="ExternalInput")
    d["id16"] = nc.dram_tensor("id16", [128, 128], F16, kind="ExternalInput")
    # raw main-conv PSUM blocks [sq*4 + hf*2 + q] = [128 out-ch, 512 pi2'];
    # the fixed pi2'->pixel permutation happens on host during unshard
    d["outr"] = nc.dram_tensor("outr", [16, 128, 512], F32, kind="ExternalOutput")

    AO = mybir.AluOpType

    with tile.TileContext(nc) as tc:
        with (
            tc.tile_pool(name="imgs", bufs=1) as imgs,
            tc.tile_pool(name="wts", bufs=1) as wts,
            tc.tile_pool(name="meta", bufs=1) as meta,
            tc.tile_pool(name="gbuf", bufs=8) as gbuf,
            tc.tile_pool(name="hbuf", bufs=2) as hbuf,
            tc.tile_pool(name="vbuf", bufs=2) as vbuf,
            tc.tile_pool(name="obuf", bufs=4) as obuf,
        ):
            # ---- load image + weights + constants (single DMAs each; order =
            #      first-use order so the conv pipeline starts ASAP)
            # loads ordered by first use on the critical path: the offsets
            # conv (xpad rows 0:10 + womb) unblocks first, big slices later
            womb = wts.tile([C, 9, 18], F16)
            nc.sync.dma_start(womb[:], d["womb"][:])
            xpad = imgs.tile([C, 66 * 66], F16)
            nc.sync.dma_start(xpad[:, 0:1188], d["xpad"][:, 0:1188])
            ob = wts.tile([18, 1], F32)
            nc.sync.dma_start(ob[:], d["ob"][:])
            id16 = wts.tile([128, 128], F16)
            nc.sync.dma_start(id16[:], d["id16"][:])
            selt = wts.tile([128, 9, 3, 128], F16)
            nc.sync.dma_start(selt[:], d["selt"][:])
            basyx = wts.tile([128, 9, KCH, 2], F32)
            nc.sync.dma_start(basyx[:], d["basyx"][:])
            nc.sync.dma_start(xpad[:, 1188:66 * 34], d["xpad"][:, 1188:66 * 34])
            wmtb = wts.tile([C, 9, 9], F16)
            nc.sync.dma_start(wmtb[:], d["wmtb"][:])
            mb = wts.tile([9, 1], F32)
            nc.sync.dma_start(mb[:], d["mb"][:])
            nc.sync.dma_start(xpad[:, 66 * 34:], d["xpad"][:, 66 * 34:])
            w2 = wts.tile([C, 9, 2, 128], F16)
            nc.sync.dma_start(w2[:], d["w2"][:])
            # junk tiles for PE p-state warmup + Act table preload (values
            # never consumed)
            junka = wts.tile([128, 128], F16)
            junkb = wts.tile([128, 512], F16)
            junkc = wts.tile([18, 4], F16)
            nc.vector.memset(junka[:], 0.0)
            nc.vector.memset(junkb[:], 0.0)
            nc.scalar.activation(junkc[:], junka[0:18, 0:4],
                                 mybir.ActivationFunctionType.Identity,
                                 bias=0.0, scale=1.0)
            nc.scalar.activation(junkc[:], junka[0:18, 0:4],
                                 mybir.ActivationFunctionType.Sigmoid,
                                 bias=0.0, scale=1.0)

            # rows 0:18 offsets, 32:41 mod (engine outputs need 32-aligned
            # partition starts; rows 18:32 stay uninitialized and are never
            # read -- the transposes contract only 0:18 / 32:41)
            ocm = meta.tile([41, PIX], F16)
            opm = meta.tile([128, KCH, 41], F16)   # pixel-major
            scal = meta.tile([128, 9, KCH, 4], F16)
            idxt = meta.tile([128, 9, KCH], I32)

            with (
                tc.tile_pool(name="psc", bufs=2, space="PSUM") as psc,
                tc.tile_pool(name="pst1", bufs=2, space="PSUM") as pst1,
            ):
                # PE p-state warmup: junk matmuls keep the PE busy from t=0 so
                # the conv matmuls run at full clock once xpad lands
                warm = psc.tile([18, 512], F32, tag="po", name="warm")
                for _ in range(4):
                    nc.tensor.matmul(warm[:], junka[:, 0:18], junkb[:],
                                     start=True, stop=True)

                # Front-end in two phases: a small first phase (conv tiles
                # 0:2, chunks 0:8) so the first gathers start early, then the
                # rest in one pass. Each phase runs the full offsets path
                # (po conv -> trA -> sel -> idx) in pass A, then the mod path
                # (pm conv -> trB -> scales) in pass B, so gathers never wait
                # on the mod conv (which needs the whole image).
                Fall = meta.tile([128, 9, KCH, 2], F32)
                for tl_lo, tl_hi, k_lo, k_hi in ((0, 1, 0, 4), (1, 2, 4, 8),
                                                 (2, 8, 8, 32)):
                    HK = k_hi - k_lo
                    ho = k_lo
                    # pass A: offsets conv
                    for tl in range(tl_lo, tl_hi):
                        po = psc.tile([18, 512], F32, tag="po")
                        for t in range(9):
                            dy, dx = t // 3, t % 3
                            rhs1 = bass.AP(
                                tensor=xpad[:].tensor,
                                offset=xpad[:].offset + dy * 66 + dx + tl * 8 * 66,
                                ap=[list(xpad[:].ap[0]), [66, 8], [1, 64]],
                            )
                            nc.tensor.matmul(po[:], womb[:, t, :], rhs1,
                                             start=(t == 0), stop=(t == 8))
                        nc.scalar.activation(
                            ocm[0:18, tl * 512:(tl + 1) * 512], po[:],
                            mybir.ActivationFunctionType.Identity,
                            bias=ob[:], scale=1.0)
                    for tl in range(tl_lo, tl_hi):
                        ptA = pst1.tile([128, 4, 18], F32, tag="ptA", bufs=1)
                        for k4 in range(4):
                            k = tl * 4 + k4
                            nc.tensor.matmul(
                                ptA[:, k4, :],
                                ocm[0:18, k * 128:(k + 1) * 128],
                                id16[0:18, 0:18], start=True, stop=True)
                        dstA = bass.AP(
                            tensor=opm[:].tensor,
                            offset=opm[:].offset + tl * 4 * 41,
                            ap=[list(opm[:].ap[0]), [41, 4], [1, 18]],
                        )
                        nc.vector.tensor_copy(dstA, ptA[:])
                    # pass A metadata: -> flat row idx (+ frac, kept for B)
                    for n2 in range(9):
                        a2 = n2 // 3
                        oyx = pst1.tile([128, HK, 2], F32, tag="sel")
                        for r in range(3):
                            ch = 3 * r + a2
                            rhs = bass.AP(
                                tensor=opm[:].tensor,
                                offset=opm[:].offset + ch + ho * 41,
                                ap=[list(opm[:].ap[0]), [41, HK], [9, 2]],
                            )
                            nc.tensor.matmul(oyx[:], selt[:, n2, r, :], rhs,
                                             start=(r == 0), stop=(r == 2))
                        P = meta.tile([128, HK, 2], F32, tag="P")
                        nc.vector.tensor_add(P[:], oyx[:],
                                             basyx[:, n2, ho:ho + HK, :])
                        nc.vector.tensor_scalar(P[:], P[:], 0.0, 63.0,
                                                AO.max, AO.min)
                        R0 = meta.tile([128, HK, 2], F32, tag="R0")
                        nc.vector.tensor_scalar(R0[:], P[:], -0.5, 12582912.0,
                                                AO.add, AO.add)
                        nc.vector.tensor_scalar_add(R0[:], R0[:], -12582912.0)
                        F = Fall[:, n2, ho:ho + HK, :]
                        nc.vector.tensor_sub(F, P[:], R0[:])
                        f00 = meta.tile([128, HK], F32, tag="f00")
                        nc.vector.scalar_tensor_tensor(
                            f00[:], R0[:, :, 1], 64.0, R0[:, :, 0], AO.mult, AO.add)
                        nc.vector.tensor_copy(idxt[:, n2, ho:ho + HK], f00[:])
                    # pass B: mod conv
                    for tl in range(tl_lo, tl_hi):
                        pm = psc.tile([9, 512], F32, tag="pm")
                        for t in range(9):
                            dy, dx = t // 3, t % 3
                            rhs2 = bass.AP(
                                tensor=xpad[:].tensor,
                                offset=xpad[:].offset + dx * 66 + dy + tl * 8,
                                ap=[list(xpad[:].ap[0]), [1, 8], [66, 64]],
                            )
                            nc.tensor.matmul(pm[:], wmtb[:, t, :], rhs2,
                                             start=(t == 0), stop=(t == 8))
                        nc.scalar.activation(
                            ocm[32:41, tl * 512:(tl + 1) * 512], pm[:],
                            mybir.ActivationFunctionType.Sigmoid,
                            bias=mb[:], scale=1.0)
                    for tl in range(tl_lo, tl_hi):
                        ptB = pst1.tile([128, 4, 9], F32, tag="ptB", bufs=1)
                        for k4 in range(4):
                            k = tl * 4 + k4
                            nc.tensor.matmul(
                                ptB[:, k4, :],
                                ocm[32:41, k * 128:(k + 1) * 128],
                                id16[32:41, 32:41], start=True, stop=True)
                        dstB = bass.AP(
                            tensor=opm[:].tensor,
                            offset=opm[:].offset + tl * 4 * 41 + 32,
                            ap=[list(opm[:].ap[0]), [41, 4], [1, 9]],
                        )
                        nc.vector.tensor_copy(dstB, ptB[:])
                    # pass B metadata: bilinear*modulation scales (fp16)
                    for n2 in range(9):
                        F = Fall[:, n2, ho:ho + HK, :]
                        mrow = opm[:, ho:ho + HK, 32 + n2]
                        v1 = meta.tile([128, HK], F32, tag="v1")
                        v0 = meta.tile([128, HK], F32, tag="v0")
                        sc4 = meta.tile([128, 4, HK], F32, tag="sc4")
                        nc.vector.tensor_mul(v1[:], mrow, F[:, :, 1])
                        nc.vector.tensor_sub(v0[:], mrow, v1[:])
                        nc.vector.tensor_mul(sc4[:, 1, :], v0[:], F[:, :, 0])
                        nc.vector.tensor_sub(sc4[:, 0, :], v0[:], sc4[:, 1, :])
                        nc.vector.tensor_mul(sc4[:, 3, :], v1[:], F[:, :, 0])
                        nc.vector.tensor_sub(sc4[:, 2, :], v1[:], sc4[:, 3, :])
                        # convert to fp16 [k, corner]-interleaved in one copy
                        csrc = bass.AP(
                            tensor=sc4[:].tensor, offset=sc4[:].offset,
                            ap=[list(sc4[:].ap[0]), [1, HK], [HK, 4]],
                        )
                        cdst = bass.AP(
                            tensor=scal[:].tensor,
                            offset=scal[:].offset + n2 * (KCH * 4) + ho * 4,
                            ap=[list(scal[:].ap[0]), [4, HK], [1, 4]],
                        )
                        nc.vector.tensor_copy(cdst, csrc)

            with (
                tc.tile_pool(name="pst", bufs=3, space="PSUM") as pst,
                tc.tile_pool(name="psm", bufs=1, space="PSUM") as psm,
            ):
                # ---- per spatial-quarter: gather + scale + reduce-transpose;
                #      main-conv matmuls interleave per n2 (PSUM accumulates
                #      while later n2 groups are still gathering)
                for sq in range(4):
                    vc = vbuf.tile([C, 9, 1024], F16, tag="vc")
                    accq = {}
                    for hf in range(2):
                        for tl2 in range(2):
                            accq[hf, tl2] = psm.tile(
                                [128, 512], F32, tag=f"mm{hf}{tl2}",
                                name=f"acc{hf}{tl2}")
                    for n2 in range(9):
                        g = gbuf.tile([128, 8, 512], F16, tag="g")
                        for kk in range(8):
                            k = sq * 8 + kk
                            dstg = bass.AP(
                                tensor=g[:].tensor,
                                offset=g[:].offset + kk * 512,
                                ap=[list(g[:].ap[0]), [1, 512]],
                            )
                            nc.gpsimd.indirect_dma_start(
                                out=dstg, out_offset=None,
                                in_=d["ptab"][:],
                                in_offset=bass.IndirectOffsetOnAxis(
                                    ap=idxt[:, n2, k:k + 1], axis=0),
                            )
                        h = hbuf.tile([128, 8, 128, 2], F16, tag="h")
                        for q in range(2):
                            gv = bass.AP(
                                tensor=g[:].tensor,
                                offset=g[:].offset + q * 2048,
                                ap=[list(g[:].ap[0]), [512, 4], [4, 128], [1, 4]],
                            )
                            sv = bass.AP(
                                tensor=scal[:].tensor,
                                offset=(scal[:].offset + n2 * (KCH * 4)
                                        + sq * 32 + q * 16),
                                ap=[list(scal[:].ap[0]), [4, 4], [0, 128], [1, 4]],
                            )
                            nc.vector.tensor_mul(gv, gv, sv)
                            ha = bass.AP(
                                tensor=g[:].tensor,
                                offset=g[:].offset + q * 2048,
                                ap=[list(g[:].ap[0]), [512, 4], [4, 128], [1, 2]],
                            )
                            hb = bass.AP(
                                tensor=g[:].tensor,
                                offset=g[:].offset + q * 2048 + 2,
                                ap=[list(g[:].ap[0]), [512, 4], [4, 128], [1, 2]],
                            )
                            hd = bass.AP(
                                tensor=h[:].tensor,
                                offset=h[:].offset + q * 1024,
                                ap=[list(h[:].ap[0]), [256, 4], [2, 128], [1, 2]],
                            )
                            nc.vector.tensor_add(hd, ha, hb)
                            acc = pst.tile([128, 512], F32, tag="tr")
                            for kk4 in range(4):
                                kk = q * 4 + kk4
                                for j in range(2):
                                    lhsT = bass.AP(
                                        tensor=h[:].tensor,
                                        offset=h[:].offset + kk * 256 + j,
                                        ap=[list(h[:].ap[0]), [2, 128]],
                                    )
                                    nc.tensor.matmul(
                                        acc[:, kk4 * 128:(kk4 + 1) * 128],
                                        lhsT, id16[:],
                                        start=(j == 0), stop=(j == 1))
                            nc.scalar.copy(vc[:, n2, q * 512:(q + 1) * 512], acc[:])
                            for hf in range(2):
                                nc.tensor.matmul(
                                    accq[hf, q][:], w2[:, n2, hf, :],
                                    vc[:, n2, q * 512:(q + 1) * 512],
                                    start=(n2 == 0), stop=(n2 == 8))

                    # store raw blocks (contiguous; host unscrambles)
                    for hf in range(2):
                        for q in range(2):
                            outq = obuf.tile([128, 512], F32, tag="oq",
                                             name="outq")
                            nc.scalar.copy(outq[:], accq[hf, q][:])
                            nc.sync.dma_start(
                                d["outr"][sq * 4 + hf * 2 + q], outq[:])

    nc.compile()
    _CACHE["nc"] = nc
    return nc


def _host_inputs(b_x, offset_w, offset_b, mod_w, mod_b, conv_w):
    hc = _build_host_constants()
    img = b_x.astype(np.float32)
    womb = np.zeros((C, 9, 18), np.float16)
    wmtb = np.zeros((C, 9, 9), np.float16)
    for t in range(9):
        dy, dx = t // 3, t % 3
        womb[:, t, :] = offset_w[:, :, dy, dx].T
        wmtb[:, 3 * dx + dy, :] = mod_w[:, :, dy, dx].T
    w2 = np.zeros((C, 9, 2, 128), np.float16)
    for n2 in range(9):
        a2, e2 = n2 // 3, n2 % 3
        for hf in range(2):
            w2[:, n2, hf, :] = conv_w[128 * hf:128 * (hf + 1), :, a2, e2].T
    return {
        "xpad": _pad66(img),
        "ptab": _patch_table(img),
        "womb": womb,
        "wmtb": wmtb,
        "ob": offset_b.reshape(18, 1).astype(np.float32),
        "mb": mod_b.reshape(9, 1).astype(np.float32),
        "selt": hc["sel"].reshape(128, 9 * 3 * 128),
        "basyx": hc["basyx"].reshape(128, 9 * KCH * 2),
        "w2": w2.reshape(C, 9 * 2 * 128),
        "id16": hc["ident16"],
    }


def kernel(x, offset_w, offset_b, mod_w, mod_b, conv_w):
    nc = _build_program()
    in_maps = [
        _host_inputs(x[b], offset_w, offset_b, mod_w, mod_b, conv_w)
        for b in range(B)
    ]
    res = run_bass_kernel_spmd(nc, in_maps, core_ids=list(range(B)))
    out = np.empty((B, OUT, H, W), np.float32)
    for b in range(B):
        # outr[sq*4 + hf*2 + q] = [128 o, 512 pi2'] with
        # pi2' = (2sq+q)*512 + q2, j2 = 8*(2sq+q) + q2//64, i2 = q2%64
        outr = res.results[b]["outr"].reshape(4, 2, 2, 128, 8, 64)
        for sq in range(4):
            for hf in range(2):
                for q in range(2):
                    j2 = 16 * sq + 8 * q
                    out[b, 128 * hf:128 * (hf + 1), :, j2:j2 + 8] = (
                        outr[sq, hf, q].transpose(0, 2, 1))
    return out


if __name__ == "__main__":
    rng = np.random.default_rng(0)
    ins = {
        "x": rng.standard_normal((B, C, H, W), dtype=np.float32),
        "offset_w": (rng.standard_normal((18, C, 3, 3)) / 34).astype(np.float32),
        "offset_b": (rng.standard_normal(18) * 0.01).astype(np.float32),
        "mod_w": (rng.standard_normal((9, C, 3, 3)) / 34).astype(np.float32),
        "mod_b": (rng.standard_normal(9) * 0.01).astype(np.float32),
        "conv_w": (rng.standard_normal((OUT, C, 3, 3)) / 34).astype(np.float32),
    }
    o = kernel(**ins)
    print("out", o.shape, o.dtype, np.abs(o).max())
